# revision 22
# baseline (speedup 1.0000x reference)
"""Trainium2 Bass kernel for nn_MultiHeadAttention_6107443495349.

Computes, per batch b (one batch per NeuronCore, 8 cores):
    q = (query @ Wq + bq)  -> heads [H, S, DK]
    k = (key   @ Wk + bk)  -> heads [H, S, DK]
    scores = q @ k^T / sqrt(DK)                     [H, S, S]
    asp = aspect @ Wd + bd                          [DK]
    aspect_scores[h, j] = tanh(asp . k[h, j] + bias_m)
    C[h, j] = mask[j] ? aspect_scores[h, j] : -1e9
    L = scores + C[row-broadcast] + short
    out = softmax(L, axis=-1)

Masked entries become -1e9 + short + scores, which underflows exp() to
exactly 0, matching the reference's exactly-0 masked probabilities.

Implementation notes:
  - Matmuls run in float32r (1 cycle/row vs 4 for fp32). ISA: f32r
    operands must be produced by a rounding compute op, lhs innermost
    free count even, dst at partition 0.
  - qT/kT per head: [64, H, S] tiles (partition = head dim d).
  - aspect rows: accumulating block-diagonal stationary [64, H] per head
    writes row h = asp . k_h, zeros elsewhere.
  - C row-broadcast over 128 query rows: rank-1 accumulating matmul with
    a ones stationary row; C rows gathered to partitions {0, 64}.
  - exp on ScalarE with fused row-sum (accum_out); softmax divide is a
    DVE tensor_tensor multiply with a free-dim-broadcast reciprocal.
"""
import numpy as np
from contextlib import ExitStack

import concourse.bass as bass
import concourse.bacc as bacc
import concourse.tile as tile
import concourse.mybir as mybir
from concourse.bass_utils import run_bass_kernel_spmd
from concourse.masks import make_identity

F32 = mybir.dt.float32
F32R = mybir.dt.float32r
I32 = mybir.dt.int32
AF = mybir.ActivationFunctionType

B, S, D, H, DK = 8, 1024, 768, 12, 64
ND = D // 128   # 6 contraction tiles
NI = S // 128   # 8 query-row tiles
NJ = S // 512   # 2 key-column halves
NH2 = H // 2    # 6 head pairs


def build_program():
    nc = bacc.Bacc("TRN2", target_bir_lowering=False, debug=False)

    query = nc.dram_tensor("query", [S, D], F32, kind="ExternalInput").ap()
    key = nc.dram_tensor("key", [S, D], F32, kind="ExternalInput").ap()
    short = nc.dram_tensor("short", [H, S, S], F32, kind="ExternalInput").ap()
    aspect = nc.dram_tensor("aspect", [D], F32, kind="ExternalInput").ap()
    mask = nc.dram_tensor("mask", [1, S], I32, kind="ExternalInput").ap()
    Wq = nc.dram_tensor("Wq", [D, D], F32, kind="ExternalInput").ap()
    bq = nc.dram_tensor("bq", [D], F32, kind="ExternalInput").ap()
    Wk = nc.dram_tensor("Wk", [D, D], F32, kind="ExternalInput").ap()
    bk = nc.dram_tensor("bk", [D], F32, kind="ExternalInput").ap()
    Wd = nc.dram_tensor("Wd", [D, DK], F32, kind="ExternalInput").ap()
    bd = nc.dram_tensor("bd", [DK], F32, kind="ExternalInput").ap()
    bias_m = nc.dram_tensor("bias_m", [1], F32, kind="ExternalInput").ap()
    out_d = nc.dram_tensor("out", [H, S, S], F32, kind="ExternalOutput").ap()

    with tile.TileContext(nc) as tc, ExitStack() as ctx:
        consts = ctx.enter_context(tc.tile_pool(name="consts", bufs=1))

        # ---- constants / small loads -------------------------------------
        ident = consts.tile([128, 128], F32)
        make_identity(nc, ident)
        # ones rows at partitions 0 and 64 (rank-1 lhs must match rhs base)
        ones_f = consts.tile([65, 128], F32)
        nc.vector.memset(ones_f, 1.0)
        ones2 = consts.tile([65, 128], F32R)
        nc.vector.tensor_copy(ones2, ones_f)

        # per-head per-partition biases [64, H]
        bq_lo = consts.tile([DK, H], F32)
        nc.gpsimd.dma_start(
            out=bq_lo,
            in_=bass.AP(tensor=bq.tensor, offset=0, ap=[[1, DK], [DK, H]]))
        bq8 = consts.tile([DK, H], F32)
        nc.scalar.mul(bq8, bq_lo, 0.125)
        bkf = consts.tile([DK, H], F32)
        nc.gpsimd.dma_start(
            out=bkf,
            in_=bass.AP(tensor=bk.tensor, offset=0, ap=[[1, DK], [DK, H]]))
        bd_sb = consts.tile([DK, 1], F32)
        nc.gpsimd.dma_start(
            out=bd_sb,
            in_=bass.AP(tensor=bd.tensor, offset=0, ap=[[1, DK], [1, 1]]))
        biasm_sb = consts.tile([H, 1], F32)
        nc.gpsimd.dma_start(
            out=biasm_sb,
            in_=bass.AP(tensor=bias_m.tensor, offset=0, ap=[[0, H], [1, 1]]))

        # mask broadcast to H partitions (int32 predicate for select)
        mi = consts.tile([H, S], I32)
        nc.gpsimd.dma_start(
            out=mi,
            in_=bass.AP(tensor=mask.tensor, offset=0, ap=[[0, H], [1, S]]))
        neg = consts.tile([H, S], F32)
        nc.vector.memset(neg, -1.0e9)

        # aspect input [128, ND] (elem (p,t) = aspect[t*128+p]) and Wd tiles
        asp_raw = consts.tile([128, ND], F32)
        nc.gpsimd.dma_start(
            out=asp_raw,
            in_=bass.AP(tensor=aspect.tensor, offset=0, ap=[[1, 128], [128, ND]]))
        asp_in = consts.tile([128, ND], F32R)
        nc.vector.tensor_copy(asp_in, asp_raw)
        wd_raw = consts.tile([128, ND, DK], F32)
        nc.gpsimd.dma_start(
            out=wd_raw,
            in_=bass.AP(tensor=Wd.tensor, offset=0,
                        ap=[[DK, 128], [DK * 128, ND], [1, DK]]))
        wd_sb = consts.tile([128, ND, DK], F32R)
        nc.vector.tensor_copy(wd_sb, wd_raw)

        # asp = aspect @ Wd + bd  -> [64, 1], then block-diagonal [64, H, H]
        asp_sb = consts.tile([DK, 1], F32)
        with tc.tile_pool(name="a_ps", bufs=1, space="PSUM") as a_ps:
            psum_a = a_ps.tile([DK, 2], F32)  # N=2: f32r dst needs even cols
            for t in range(ND):
                nc.tensor.matmul(psum_a, wd_sb[:, t, :],
                                 asp_in[:, t:t + 1].broadcast_to([128, 2]),
                                 start=(t == 0), stop=(t == ND - 1))
            nc.scalar.activation(asp_sb, psum_a[:, 0:1], AF.Identity,
                                 bias=bd_sb, scale=1.0)
        asp_bdf = consts.tile([DK, H, H], F32)
        nc.vector.memset(asp_bdf, 0.0)
        for h in range(H):
            nc.vector.tensor_copy(asp_bdf[:, h, h:h + 1], asp_sb)
        asp_bd = consts.tile([DK, H, H], F32R)
        nc.vector.tensor_copy(asp_bd, asp_bdf)

        # ---- per-head aspect rows C --------------------------------------
        ct_t = consts.tile([H, S], F32)
        c_all = consts.tile([H, S], F32)
        c_allr = consts.tile([H, S], F32R)
        crow = consts.tile([65, NH2, S], F32R)  # heads 0-5 @p0, 6-11 @p64

        qtil = consts.tile([DK, H, S], F32R)
        ktil = consts.tile([DK, H, S], F32R)

        with tc.tile_pool(name="weights", bufs=1) as weights, \
             tc.tile_pool(name="xt", bufs=1) as xt_pool, \
             tc.tile_pool(name="raw", bufs=3) as raw_pool, \
             tc.tile_pool(name="tp_ps", bufs=2, space="PSUM") as tp_ps, \
             tc.tile_pool(name="proj_ps", bufs=3, space="PSUM") as proj_ps, \
             tc.tile_pool(name="ct_ps", bufs=1, space="PSUM") as ct_ps:
            psum_ct = ct_ps.tile([H, S], F32)

            def load_w(w_ap):
                w_sb = weights.tile([128, ND, D], F32R, tag="w")
                for t in range(ND):
                    wraw = raw_pool.tile([128, D], F32, tag="raw")
                    nc.sync.dma_start(out=wraw,
                                      in_=w_ap[t * 128:(t + 1) * 128, :])
                    nc.vector.tensor_copy(w_sb[:, t, :], wraw)
                return w_sb

            def transpose_in(x_ap):
                xT = xt_pool.tile([128, ND, S], F32R, tag="xT")
                for i in range(NI):
                    raw = raw_pool.tile([128, D], F32, tag="raw")
                    nc.sync.dma_start(out=raw, in_=x_ap[i * 128:(i + 1) * 128, :])
                    for t in range(ND):
                        pt = tp_ps.tile([128, 128], F32, tag="tp")
                        nc.tensor.transpose(pt, raw[:, t * 128:(t + 1) * 128], ident)
                        nc.vector.tensor_copy(xT[:, t, i * 128:(i + 1) * 128], pt)
                return xT

            def project_head(w_sb, xT, h, dest, bias_ap, scale):
                for jh in range(NJ):
                    pp = proj_ps.tile([DK, 512], F32, tag="pp")
                    for t in range(ND):
                        nc.tensor.matmul(
                            pp,
                            w_sb[:, t, h * DK:(h + 1) * DK],
                            xT[:, t, jh * 512:(jh + 1) * 512],
                            start=(t == 0), stop=(t == ND - 1))
                    nc.scalar.activation(
                        dest[:, h, jh * 512:(jh + 1) * 512],
                        pp, AF.Identity,
                        bias=bias_ap[:, h:h + 1], scale=scale)

            # ---- key path: transpose, project, aspect rows (desc heads) --
            wk_sb = load_w(Wk)
            kT = transpose_in(key)
            for h in range(H - 1, -1, -1):
                project_head(wk_sb, kT, h, ktil, bkf, 1.0)
                # accumulate row h = asp . k_h (other rows add zeros)
                for jh in range(NJ):
                    nc.tensor.matmul(
                        psum_ct[:, jh * 512:(jh + 1) * 512],
                        asp_bd[:, h, :],
                        ktil[:, h, jh * 512:(jh + 1) * 512],
                        start=(h == H - 1), stop=(h == 0))
            nc.scalar.activation(ct_t, psum_ct, AF.Tanh, bias=biasm_sb,
                                 scale=1.0)
            nc.vector.tensor_copy(c_all, neg)
            nc.vector.copy_predicated(c_all, mi, ct_t)
            nc.vector.tensor_copy(c_allr, c_all)
            # gather C rows: heads 0-5 -> partition 0, heads 6-11 -> 64
            nc.gpsimd.dma_start(out=crow[0:1, :, :], in_=c_allr[0:NH2, :])
            nc.gpsimd.dma_start(out=crow[64:65, :, :], in_=c_allr[NH2:H, :])

            # ---- query path (desc heads, matches main loop order) --------
            wq_sb = load_w(Wq)
            qT = transpose_in(query)
            for h in range(H - 1, -1, -1):
                project_head(wq_sb, qT, h, qtil, bq8, 0.125)

        # ---- main attention loop -----------------------------------------
        with tc.tile_pool(name="s_ps", bufs=3, space="PSUM") as s_ps, \
             tc.tile_pool(name="sh", bufs=5) as sh_pool, \
             tc.tile_pool(name="st", bufs=2) as st_pool, \
             tc.tile_pool(name="ex", bufs=2) as ex_pool, \
             tc.tile_pool(name="ot", bufs=3) as ot_pool, \
             tc.tile_pool(name="rs", bufs=6) as rs_pool:
            for h in range(H - 1, -1, -1):
                cb = 0 if h < NH2 else 64
                for i in range(NI):
                    ps_s = s_ps.tile([128, S], F32, tag="scores")
                    for jh in range(NJ):
                        nc.tensor.matmul(
                            ps_s[:, jh * 512:(jh + 1) * 512],
                            qtil[:, h, i * 128:(i + 1) * 128],
                            ktil[:, h, jh * 512:(jh + 1) * 512],
                            start=True, stop=False)
                        nc.tensor.matmul(
                            ps_s[:, jh * 512:(jh + 1) * 512],
                            ones2[cb:cb + 1, :],
                            crow[cb:cb + 1, h % NH2, jh * 512:(jh + 1) * 512],
                            start=False, stop=True)
                    sh = sh_pool.tile([128, S], F32, tag="sh")
                    nc.sync.dma_start(out=sh,
                                      in_=short[h, i * 128:(i + 1) * 128, :])
                    st = st_pool.tile([128, S], F32, tag="st")
                    nc.vector.tensor_add(st, ps_s, sh)
                    ex = ex_pool.tile([128, S], F32, tag="ex")
                    rs = rs_pool.tile([128, 1], F32, tag="rs")
                    nc.scalar.activation(ex, st, AF.Exp, accum_out=rs)
                    rc = rs_pool.tile([128, 1], F32, tag="rc")
                    nc.vector.reciprocal(rc, rs)
                    ot = ot_pool.tile([128, S], F32, tag="ot")
                    nc.vector.tensor_mul(ot, ex, rc.broadcast_to([128, S]))
                    nc.sync.dma_start(out=out_d[h, i * 128:(i + 1) * 128, :],
                                      in_=ot)
    nc.compile()
    return nc


_NC = None


def _get_nc():
    global _NC
    if _NC is None:
        _NC = build_program()
    return _NC


def _in_maps(inputs):
    q = np.ascontiguousarray(inputs["query"], np.float32)
    k = np.ascontiguousarray(inputs["key_"], np.float32)
    sh = np.ascontiguousarray(inputs["short"], np.float32)
    asp = np.ascontiguousarray(inputs["aspect"], np.float32)
    m = np.ascontiguousarray(inputs["mask"], np.int32)
    rep = {
        "Wq": np.ascontiguousarray(inputs["Wq"], np.float32),
        "bq": np.ascontiguousarray(inputs["bq"], np.float32),
        "Wk": np.ascontiguousarray(inputs["Wk"], np.float32),
        "bk": np.ascontiguousarray(inputs["bk"], np.float32),
        "Wd": np.ascontiguousarray(inputs["Wd"], np.float32),
        "bd": np.ascontiguousarray(inputs["bd"], np.float32),
        "bias_m": np.ascontiguousarray(inputs["bias_m"], np.float32),
    }
    return [
        dict(query=q[b], key=k[b], short=sh[b], aspect=asp[b], mask=m[b], **rep)
        for b in range(B)
    ]


def run_on_hw(inputs, trace=False):
    nc = _get_nc()
    res = run_bass_kernel_spmd(nc, _in_maps(inputs), core_ids=list(range(B)),
                               trace=trace)
    out = np.stack([res.results[b]["out"] for b in range(B)])
    return out, res.exec_time_ns


def kernel(**inputs) -> np.ndarray:
    out, _ = run_on_hw(inputs, trace=False)
    return out
